# revision 26
# baseline (speedup 1.0000x reference)
"""Additive (Bahdanau) attention weights kernel for Trainium2, 8 NeuronCores.

Problem: nn_AdditiveAttention_5798205849844
  queries [4, 256, 256] f32, keys [4, 512, 256] f32, values (unused),
  mask [4, 256, 512] bool, W_concat [256, 512], b_concat [256],
  W_logit [1, 256], b_logit [1].
  out = softmax_k( sum_e w[e] * tanh(qp[b,q,e] + kp[b,k,e]) , masked ) -> [4, 256, 512]

Sharding: data-parallel over the 1024 (b, q) rows -> 8 cores x 128 rows.
Each core gets its batch's full keys + replicated params; outputs are disjoint.

Per-core algorithm (ScalarE-bound):
  qpT[e,q] = Wq @ q^T + b_concat   (PE matmuls on transposed operands)
  kpT[e,k] = Wk @ k^T              (kept resident in PSUM)
  for each q row:  t[e,k] = tanh(kpT[e,k] + qpT[e,q])   <- one ACTIVATE per
      (q, e-half): the per-partition bias operand does the outer add for free
  logits[q,k] = w_logit^T t        (PE matmul, [128,1] stationary -> [1,512] rows)
  masked softmax over k on DVE (exact parity with the reference's
  fully-masked-row un-masking rule).
"""
import sys

sys.path.insert(0, "/opt/trn_rl_repo")

import numpy as np

import concourse.bass as bass
import concourse.tile as tile
from concourse import mybir
from concourse.bass_utils import run_bass_kernel_spmd

F32 = mybir.dt.float32
F16 = mybir.dt.float16
U8 = mybir.dt.uint8
AF = mybir.ActivationFunctionType
ALU = mybir.AluOpType

B, LQ, LKV, D = 4, 256, 512, 256
NCORES = 8
QSH = (B * LQ) // NCORES  # 128 query rows per core
ET = D // 128  # e-tiles (output dim of W blocks)
DT = D // 128  # d-tiles (contraction dim)
KT = LKV // 128  # k-tiles


def _split_multiwait(nc, maxw=1):
    """Walrus here rejects >1 sync-wait per instruction (Too many sync wait
    commands on the Tile tail drain). Move overflow waits onto preceding
    same-engine NOPs; sequential execution preserves the sync semantics."""
    for f in nc.m.functions:
        for blk in f.blocks:
            new = []
            for inst in blk.instructions:
                si = inst.sync_info
                if si is not None and len(si.on_wait) > maxw:
                    waits = list(si.on_wait)
                    overflow, keep = waits[:-maxw], waits[-maxw:]
                    for i in range(0, len(overflow), maxw):
                        new.append(
                            mybir.InstNoOp(
                                name=f"{inst.name}-sw{i}",
                                engine=inst.engine,
                                ins=[],
                                outs=[],
                                sync_info=mybir.SyncInfo(
                                    on_wait=overflow[i : i + maxw], on_update=[]
                                ),
                            )
                        )
                    si.on_wait = keep
                new.append(inst)
            blk.instructions[:] = new


def _build_program():
    from contextlib import ExitStack

    nc = bass.Bass(name="additive_attn")
    # all matrix operands arrive pre-transposed (d-major) from the host
    qT_sh = nc.dram_tensor("qT_sh", [D, QSH], F16, kind="ExternalInput")
    kT_full = nc.dram_tensor("kT_full", [D, LKV], F16, kind="ExternalInput")
    mask_sh = nc.dram_tensor("mask_sh", [QSH, LKV], U8, kind="ExternalInput")
    wqT_d = nc.dram_tensor("wqT_d", [D, D], F16, kind="ExternalInput")
    wkT_d = nc.dram_tensor("wkT_d", [D, D], F16, kind="ExternalInput")
    b_cat = nc.dram_tensor("b_cat", [D, 1], F32, kind="ExternalInput")
    w_log = nc.dram_tensor("w_log", [D, 1], F32, kind="ExternalInput")
    out_w = nc.dram_tensor("out_w", [QSH, LKV], F32, kind="ExternalOutput")

    with tile.TileContext(nc) as tc:
        with ExitStack() as ctx:
            const = ctx.enter_context(tc.tile_pool(name="const", bufs=1))
            work = ctx.enter_context(tc.tile_pool(name="work", bufs=1))
            tpool = ctx.enter_context(tc.tile_pool(name="tanh", bufs=24))
            ps_kpt = ctx.enter_context(tc.tile_pool(name="ps_kpt", bufs=1, space="PSUM"))
            ps_row = ctx.enter_context(tc.tile_pool(name="ps_row", bufs=6, space="PSUM"))
            rowsb = ctx.enter_context(tc.tile_pool(name="rowsb", bufs=6))

            # preload the tanh/exp activation table set immediately so the
            # ACT engine is ready the moment kpT lands (a late table load
            # stalls ACT, idles the PE >3.4us, and HAM re-throttles it to
            # 1.2GHz for the rest of the kernel).
            warm = const.tile([128, 1], F32, tag="warm")
            nc.vector.memset(warm, 0.0)
            warm2 = const.tile([128, 1], F32, tag="warm2")
            nc.scalar.activation(out=warm2, in_=warm, func=AF.Tanh)

            # PE warmup: ~3.4us of back-to-back matmuls on memset tiles fill
            # one HAM SHORT window while the input DMAs stream, so the setup
            # matmuls (and the first loop groups) run at 2.4GHz instead of
            # 1.2GHz, and the PE's post-preamble dispatch latency is hidden.
            wsrc = const.tile([128, LKV], F16, tag="wsrc")
            nc.vector.memset(wsrc, 0.0)
            wst = const.tile([128, 1], F16, tag="wst")
            nc.vector.memset(wst, 0.0)
            ps_warm = ps_row.tile([1, LKV], F32, tag="row", name="warmrow")
            for _ in range(8):
                nc.tensor.matmul(ps_warm, wst, wsrc, start=True, stop=True)

            # ---- loads (operands pre-transposed on host) ---------------
            # wT[d, which, dt, e]: which 0 -> WqT, 1 -> WkT
            wT = const.tile([128, 2, DT, D], F16, tag="wT")
            # kp (= WkT.T @ kT) gates the first tanh: its operands load first,
            # on two parallel queues (WkT+WqT on sync, kT on scalar-HWDGE);
            # qT rides the gpsimd queue so it never queues behind them.
            for dt in range(DT):
                nc.sync.dma_start(
                    out=wT[:, 1, dt, :], in_=wkT_d[dt * 128 : (dt + 1) * 128, :]
                )
            kTt = const.tile([128, DT, LKV], F16, tag="kTt")
            for dt in range(DT):
                for half in range(2):
                    nc.scalar.dma_start(
                        out=kTt[:, dt, half * 256 : (half + 1) * 256],
                        in_=kT_full[
                            dt * 128 : (dt + 1) * 128, half * 256 : (half + 1) * 256
                        ],
                    )
            for dt in range(DT):
                nc.sync.dma_start(
                    out=wT[:, 0, dt, :], in_=wqT_d[dt * 128 : (dt + 1) * 128, :]
                )
            qT = const.tile([128, DT, QSH], F16, tag="qT")
            for dt in range(DT):
                nc.gpsimd.dma_start(
                    out=qT[:, dt, :], in_=qT_sh[dt * 128 : (dt + 1) * 128, :]
                )
            wl_sb = const.tile([128, ET], F32, tag="wl_sb")
            for et in range(ET):
                nc.gpsimd.dma_start(
                    out=wl_sb[:, et : et + 1], in_=w_log[et * 128 : (et + 1) * 128, :]
                )
            # fp16 copy of w_logit: fp32 matmuls run LOW_HIGH double-pass on
            # the PE (4x the cost); fp16 keeps 10 mantissa bits at bf16 speed.
            wl_16 = const.tile([128, ET], F16, tag="wl_16")
            nc.vector.tensor_copy(out=wl_16, in_=wl_sb)
            b_sb = const.tile([128, ET], F32, tag="b_sb")
            for et in range(ET):
                nc.gpsimd.dma_start(
                    out=b_sb[:, et : et + 1], in_=b_cat[et * 128 : (et + 1) * 128, :]
                )
            mask_sb = const.tile([128, LKV], U8, tag="mask_sb")
            nc.gpsimd.dma_start(out=mask_sb, in_=mask_sh[:, :])

            # ---- kpT (PSUM-resident) and qpT, et-interleaved so the
            # et=0 pair (which gates the first tanh) completes first ------
            qpT = const.tile([128, ET, QSH], F32, tag="qpT")
            kpt = []
            for et in range(ET):
                kp = ps_kpt.tile([128, LKV], F32, tag=f"kpt{et}")
                for dt in range(DT):
                    nc.tensor.matmul(
                        kp,
                        wT[:, 1, dt, et * 128 : (et + 1) * 128],
                        kTt[:, dt, :],
                        start=(dt == 0),
                        stop=(dt == DT - 1),
                    )
                kpt.append(kp)
                ps = ps_row.tile([128, 128], F32, tag="row", name=f"qp_ps{et}")
                for dt in range(DT):
                    nc.tensor.matmul(
                        ps,
                        wT[:, 0, dt, et * 128 : (et + 1) * 128],
                        qT[:, dt, :],
                        start=(dt == 0),
                        stop=(dt == DT - 1),
                    )
                nc.scalar.activation(
                    out=qpT[:, et, :],
                    in_=ps,
                    func=AF.Identity,
                    bias=b_sb[:, et : et + 1],
                    scale=1.0,
                )

            # ---- main loop: tanh + weighted reduce --------------------
            logits = const.tile([128, LKV], F32, tag="logits")
            # groups of 4 q-rows: 4 same-stationary matmuls run back-to-back
            # per LDWEIGHTS, so the PE pipelines fill/drain even when the HAM
            # clock-gate has it at 1.2GHz (alternating stationaries per MM
            # serialize at the isolated-MM latency and the PE falls behind).
            GRP = 4
            for qg in range(0, QSH, GRP):
                ts = []
                for q in range(qg, qg + GRP):
                    pair = []
                    for et in range(ET):
                        t_t = tpool.tile([128, LKV], F16, tag=f"t{et}")
                        nc.scalar.activation(
                            out=t_t,
                            in_=kpt[et],
                            func=AF.Tanh,
                            bias=qpT[:, et, q : q + 1],
                            scale=1.0,
                        )
                        pair.append(t_t)
                    ts.append(pair)
                rows = [
                    ps_row.tile([1, LKV], F32, tag="row", name=f"row{qg}_{g}")
                    for g in range(GRP)
                ]
                for et in range(ET):
                    for g in range(GRP):
                        nc.tensor.matmul(
                            rows[g],
                            wl_16[:, et : et + 1],
                            ts[g][et],
                            start=(et == 0),
                            stop=(et == ET - 1),
                        )
                for g, q in enumerate(range(qg, qg + GRP)):
                    rsb = rowsb.tile([1, LKV], F32, tag="rowsb")
                    nc.vector.tensor_copy(out=rsb, in_=rows[g])
                    nc.sync.dma_start(out=logits[q : q + 1, :], in_=rsb)

            # ---- masked softmax over k (two 64-row halves: the first
            # half runs while the main loop is still streaming) -----------
            maskf = work.tile([128, LKV], F32, tag="maskf")
            nc.vector.tensor_copy(out=maskf, in_=mask_sb)
            # reference un-masking rule, applied upfront where it hides under
            # the tanh stream: a fully-masked row attends everything
            # (maskf := maskf OR row-is-all-zero).
            rowmax = work.tile([128, 1], F32, tag="rowmax")
            nc.vector.tensor_reduce(
                out=rowmax, in_=maskf, axis=mybir.AxisListType.X, op=ALU.max
            )
            flagm = work.tile([128, 1], F32, tag="flagm")
            nc.vector.tensor_scalar(
                out=flagm, in0=rowmax, scalar1=0.0, scalar2=None, op0=ALU.is_equal
            )
            nc.vector.tensor_scalar_max(out=maskf, in0=maskf, scalar1=flagm)
            outw = work.tile([128, LKV], F32, tag="outw")
            for h in range(2):
                r0, r1 = h * 64, (h + 1) * 64
                expv = work.tile([128, LKV], F32, tag=f"expv{h}")
                nc.scalar.activation(
                    out=expv[r0:r1], in_=logits[r0:r1], func=AF.Exp
                )
                masked = work.tile([128, LKV], F32, tag=f"masked{h}")
                denom = work.tile([128, 1], F32, tag=f"denom{h}")
                nc.vector.scalar_tensor_tensor(
                    out=masked[r0:r1], in0=expv[r0:r1], scalar=0.0,
                    in1=maskf[r0:r1], op0=ALU.add, op1=ALU.mult,
                    accum_out=denom[r0:r1],
                )
                recip = work.tile([128, 1], F32, tag=f"recip{h}")
                nc.vector.reciprocal(out=recip[r0:r1], in_=denom[r0:r1])
                nc.vector.tensor_scalar_mul(
                    out=outw[r0:r1], in0=masked[r0:r1], scalar1=recip[r0:r1]
                )
                nc.sync.dma_start(out=out_w[r0:r1, :], in_=outw[r0:r1])

    _split_multiwait(nc)
    return nc


def _run(inputs, trace=False):
    queries = np.asarray(inputs["queries"], dtype=np.float32)
    keys = np.asarray(inputs["keys"], dtype=np.float32)
    mask = np.asarray(inputs["mask"]).astype(np.uint8)
    W_concat = np.asarray(inputs["W_concat"], dtype=np.float32)
    b_concat = np.asarray(inputs["b_concat"], dtype=np.float32)
    W_logit = np.asarray(inputs["W_logit"], dtype=np.float32)

    nc = _build_program()

    halves = NCORES // B  # 2
    wqT_d = np.ascontiguousarray(W_concat[:, :D].T.astype(np.float16))
    wkT_d = np.ascontiguousarray(W_concat[:, D:].T.astype(np.float16))
    b_cat = np.ascontiguousarray(b_concat.reshape(D, 1))
    w_log = np.ascontiguousarray(W_logit.reshape(D, 1))
    in_maps = []
    for c in range(NCORES):
        b, h = divmod(c, halves)
        in_maps.append(
            {
                "qT_sh": np.ascontiguousarray(queries[b, h * QSH : (h + 1) * QSH].T.astype(np.float16)),
                "kT_full": np.ascontiguousarray(keys[b].T.astype(np.float16)),
                "mask_sh": np.ascontiguousarray(mask[b, h * QSH : (h + 1) * QSH]),
                "wqT_d": wqT_d,
                "wkT_d": wkT_d,
                "b_cat": b_cat,
                "w_log": w_log,
            }
        )

    res = run_bass_kernel_spmd(
        nc, in_maps, core_ids=list(range(NCORES)), trace=trace
    )
    outs = [res.results[c]["out_w"] for c in range(NCORES)]
    full = np.concatenate(outs, axis=0).reshape(B, LQ, LKV)
    return full, res


def kernel(**inputs) -> np.ndarray:
    out, _ = _run(inputs, trace=False)
    return out


# revision 28
# speedup vs baseline: 1.0003x; 1.0003x over previous
"""Additive (Bahdanau) attention weights kernel for Trainium2, 8 NeuronCores.

Problem: nn_AdditiveAttention_5798205849844
  queries [4, 256, 256] f32, keys [4, 512, 256] f32, values (unused),
  mask [4, 256, 512] bool, W_concat [256, 512], b_concat [256],
  W_logit [1, 256], b_logit [1].
  out = softmax_k( sum_e w[e] * tanh(qp[b,q,e] + kp[b,k,e]) , masked ) -> [4, 256, 512]

Sharding: data-parallel over the 1024 (b, q) rows -> 8 cores x 128 rows.
Each core gets its batch's full keys + replicated params; outputs are disjoint.

Per-core algorithm (ScalarE-bound):
  qpT[e,q] = Wq @ q^T + b_concat   (PE matmuls on transposed operands)
  kpT[e,k] = Wk @ k^T              (kept resident in PSUM)
  for each q row:  t[e,k] = tanh(kpT[e,k] + qpT[e,q])   <- one ACTIVATE per
      (q, e-half): the per-partition bias operand does the outer add for free
  logits[q,k] = w_logit^T t        (PE matmul, [128,1] stationary -> [1,512] rows)
  masked softmax over k on DVE (exact parity with the reference's
  fully-masked-row un-masking rule).
"""
import sys

sys.path.insert(0, "/opt/trn_rl_repo")

import numpy as np

import concourse.bass as bass
import concourse.tile as tile
from concourse import mybir
from concourse.bass_utils import run_bass_kernel_spmd

F32 = mybir.dt.float32
F16 = mybir.dt.float16
U8 = mybir.dt.uint8
AF = mybir.ActivationFunctionType
ALU = mybir.AluOpType

B, LQ, LKV, D = 4, 256, 512, 256
NCORES = 8
QSH = (B * LQ) // NCORES  # 128 query rows per core
ET = D // 128  # e-tiles (output dim of W blocks)
DT = D // 128  # d-tiles (contraction dim)
KT = LKV // 128  # k-tiles


def _split_multiwait(nc, maxw=1):
    """Walrus here rejects >1 sync-wait per instruction (Too many sync wait
    commands on the Tile tail drain). Move overflow waits onto preceding
    same-engine NOPs; sequential execution preserves the sync semantics."""
    for f in nc.m.functions:
        for blk in f.blocks:
            new = []
            for inst in blk.instructions:
                si = inst.sync_info
                if si is not None and len(si.on_wait) > maxw:
                    waits = list(si.on_wait)
                    overflow, keep = waits[:-maxw], waits[-maxw:]
                    for i in range(0, len(overflow), maxw):
                        new.append(
                            mybir.InstNoOp(
                                name=f"{inst.name}-sw{i}",
                                engine=inst.engine,
                                ins=[],
                                outs=[],
                                sync_info=mybir.SyncInfo(
                                    on_wait=overflow[i : i + maxw], on_update=[]
                                ),
                            )
                        )
                    si.on_wait = keep
                new.append(inst)
            blk.instructions[:] = new


def _build_program():
    from contextlib import ExitStack

    nc = bass.Bass(name="additive_attn")
    # all matrix operands arrive pre-transposed (d-major) from the host
    qT_sh = nc.dram_tensor("qT_sh", [D, QSH], F16, kind="ExternalInput")
    kT_full = nc.dram_tensor("kT_full", [D, LKV], F16, kind="ExternalInput")
    mask_sh = nc.dram_tensor("mask_sh", [QSH, LKV], U8, kind="ExternalInput")
    wqT_d = nc.dram_tensor("wqT_d", [D, D], F16, kind="ExternalInput")
    wkT_d = nc.dram_tensor("wkT_d", [D, D], F16, kind="ExternalInput")
    b_cat = nc.dram_tensor("b_cat", [D, 1], F32, kind="ExternalInput")
    w_log = nc.dram_tensor("w_log", [D, 1], F32, kind="ExternalInput")
    out_w = nc.dram_tensor("out_w", [QSH, LKV], F32, kind="ExternalOutput")

    with tile.TileContext(nc) as tc:
        with ExitStack() as ctx:
            const = ctx.enter_context(tc.tile_pool(name="const", bufs=1))
            work = ctx.enter_context(tc.tile_pool(name="work", bufs=1))
            tpool = ctx.enter_context(tc.tile_pool(name="tanh", bufs=24))
            ps_kpt = ctx.enter_context(tc.tile_pool(name="ps_kpt", bufs=1, space="PSUM"))
            ps_row = ctx.enter_context(tc.tile_pool(name="ps_row", bufs=6, space="PSUM"))
            rowsb = ctx.enter_context(tc.tile_pool(name="rowsb", bufs=6))

            # preload the tanh/exp activation table set immediately so the
            # ACT engine is ready the moment kpT lands (a late table load
            # stalls ACT, idles the PE >3.4us, and HAM re-throttles it to
            # 1.2GHz for the rest of the kernel).
            warm = const.tile([128, 1], F32, tag="warm")
            nc.vector.memset(warm, 0.0)
            warm2 = const.tile([128, 1], F32, tag="warm2")
            nc.scalar.activation(out=warm2, in_=warm, func=AF.Tanh)

            # PE warmup: ~3.4us of back-to-back matmuls on memset tiles fill
            # one HAM SHORT window while the input DMAs stream, so the setup
            # matmuls (and the first loop groups) run at 2.4GHz instead of
            # 1.2GHz, and the PE's post-preamble dispatch latency is hidden.
            wsrc = const.tile([128, LKV], F16, tag="wsrc")
            nc.vector.memset(wsrc, 0.0)
            wst = const.tile([128, 1], F16, tag="wst")
            nc.vector.memset(wst, 0.0)
            ps_warm = ps_row.tile([1, LKV], F32, tag="row", name="warmrow")
            for _ in range(8):
                nc.tensor.matmul(ps_warm, wst, wsrc, start=True, stop=True)

            # ---- loads (operands pre-transposed on host) ---------------
            # wT[d, which, dt, e]: which 0 -> WqT, 1 -> WkT
            wT = const.tile([128, 2, DT, D], F16, tag="wT")
            # kp (= WkT.T @ kT) gates the first tanh: its operands load first,
            # on two parallel queues (WkT+WqT on sync, kT on scalar-HWDGE);
            # qT rides the gpsimd queue so it never queues behind them.
            for dt in range(DT):
                nc.sync.dma_start(
                    out=wT[:, 1, dt, :], in_=wkT_d[dt * 128 : (dt + 1) * 128, :]
                )
            kTt = const.tile([128, DT, LKV], F16, tag="kTt")
            for dt in range(DT):
                for half in range(2):
                    nc.scalar.dma_start(
                        out=kTt[:, dt, half * 256 : (half + 1) * 256],
                        in_=kT_full[
                            dt * 128 : (dt + 1) * 128, half * 256 : (half + 1) * 256
                        ],
                    )
            for dt in range(DT):
                nc.sync.dma_start(
                    out=wT[:, 0, dt, :], in_=wqT_d[dt * 128 : (dt + 1) * 128, :]
                )
            qT = const.tile([128, DT, QSH], F16, tag="qT")
            for dt in range(DT):
                nc.gpsimd.dma_start(
                    out=qT[:, dt, :], in_=qT_sh[dt * 128 : (dt + 1) * 128, :]
                )
            wl_sb = const.tile([128, ET], F32, tag="wl_sb")
            for et in range(ET):
                nc.gpsimd.dma_start(
                    out=wl_sb[:, et : et + 1], in_=w_log[et * 128 : (et + 1) * 128, :]
                )
            # fp16 copy of w_logit: fp32 matmuls run LOW_HIGH double-pass on
            # the PE (4x the cost); fp16 keeps 10 mantissa bits at bf16 speed.
            wl_16 = const.tile([128, ET], F16, tag="wl_16")
            nc.vector.tensor_copy(out=wl_16, in_=wl_sb)
            b_sb = const.tile([128, ET], F32, tag="b_sb")
            for et in range(ET):
                nc.gpsimd.dma_start(
                    out=b_sb[:, et : et + 1], in_=b_cat[et * 128 : (et + 1) * 128, :]
                )
            mask_sb = const.tile([128, LKV], U8, tag="mask_sb")
            nc.gpsimd.dma_start(out=mask_sb, in_=mask_sh[:, :])

            # ---- kpT (PSUM-resident) and qpT, et-interleaved so the
            # et=0 pair (which gates the first tanh) completes first ------
            qpT = const.tile([128, ET, QSH], F32, tag="qpT")
            kpt = []
            for et in range(ET):
                kp = ps_kpt.tile([128, LKV], F32, tag=f"kpt{et}")
                for dt in range(DT):
                    nc.tensor.matmul(
                        kp,
                        wT[:, 1, dt, et * 128 : (et + 1) * 128],
                        kTt[:, dt, :],
                        start=(dt == 0),
                        stop=(dt == DT - 1),
                    )
                kpt.append(kp)
                ps = ps_row.tile([128, 128], F32, tag="row", name=f"qp_ps{et}")
                for dt in range(DT):
                    nc.tensor.matmul(
                        ps,
                        wT[:, 0, dt, et * 128 : (et + 1) * 128],
                        qT[:, dt, :],
                        start=(dt == 0),
                        stop=(dt == DT - 1),
                    )
                nc.scalar.activation(
                    out=qpT[:, et, :],
                    in_=ps,
                    func=AF.Identity,
                    bias=b_sb[:, et : et + 1],
                    scale=1.0,
                )

            # ---- main loop: tanh + weighted reduce --------------------
            logits = const.tile([128, LKV], F32, tag="logits")
            # groups of 4 q-rows: 4 same-stationary matmuls run back-to-back
            # per LDWEIGHTS, so the PE pipelines fill/drain even when the HAM
            # clock-gate has it at 1.2GHz (alternating stationaries per MM
            # serialize at the isolated-MM latency and the PE falls behind).
            GRP = 4
            for qg in range(0, QSH, GRP):
                ts = []
                for q in range(qg, qg + GRP):
                    pair = []
                    for et in range(ET):
                        t_t = tpool.tile([128, LKV], F16, tag=f"t{et}")
                        nc.scalar.activation(
                            out=t_t,
                            in_=kpt[et],
                            func=AF.Tanh,
                            bias=qpT[:, et, q : q + 1],
                            scale=1.0,
                        )
                        pair.append(t_t)
                    ts.append(pair)
                rows = [
                    ps_row.tile([1, LKV], F32, tag="row", name=f"row{qg}_{g}")
                    for g in range(GRP)
                ]
                for et in range(ET):
                    for g in range(GRP):
                        nc.tensor.matmul(
                            rows[g],
                            wl_16[:, et : et + 1],
                            ts[g][et],
                            start=(et == 0),
                            stop=(et == ET - 1),
                        )
                for g, q in enumerate(range(qg, qg + GRP)):
                    rsb = rowsb.tile([1, LKV], F32, tag="rowsb")
                    nc.vector.tensor_copy(out=rsb, in_=rows[g])
                    nc.sync.dma_start(out=logits[q : q + 1, :], in_=rsb)

            # ---- masked softmax over k (two 64-row halves: the first
            # half runs while the main loop is still streaming) -----------
            maskf = work.tile([128, LKV], F32, tag="maskf")
            nc.vector.tensor_copy(out=maskf, in_=mask_sb)
            # reference un-masking rule, applied upfront where it hides under
            # the tanh stream: a fully-masked row attends everything
            # (maskf := maskf OR row-is-all-zero).
            rowmax = work.tile([128, 1], F32, tag="rowmax")
            nc.vector.tensor_reduce(
                out=rowmax, in_=maskf, axis=mybir.AxisListType.X, op=ALU.max
            )
            flagm = work.tile([128, 1], F32, tag="flagm")
            nc.vector.tensor_scalar(
                out=flagm, in0=rowmax, scalar1=0.0, scalar2=None, op0=ALU.is_equal
            )
            nc.vector.tensor_scalar_max(out=maskf, in0=maskf, scalar1=flagm)
            outw = work.tile([128, LKV], F32, tag="outw")
            for h in range(2):
                r0, r1 = h * 64, (h + 1) * 64
                expv = work.tile([128, LKV], F32, tag=f"expv{h}")
                nc.scalar.activation(
                    out=expv[r0:r1], in_=logits[r0:r1], func=AF.Exp
                )
                masked = work.tile([128, LKV], F32, tag=f"masked{h}")
                denom = work.tile([128, 1], F32, tag=f"denom{h}")
                nc.vector.scalar_tensor_tensor(
                    out=masked[r0:r1], in0=expv[r0:r1], scalar=0.0,
                    in1=maskf[r0:r1], op0=ALU.add, op1=ALU.mult,
                    accum_out=denom[r0:r1],
                )
                recip = work.tile([128, 1], F32, tag=f"recip{h}")
                nc.vector.reciprocal(out=recip[r0:r1], in_=denom[r0:r1])
                nc.vector.tensor_scalar_mul(
                    out=outw[r0:r1], in0=masked[r0:r1], scalar1=recip[r0:r1]
                )
                nc.sync.dma_start(out=out_w[r0:r1, :], in_=outw[r0:r1])

    _split_multiwait(nc)
    return nc


def _run(inputs, trace=False):
    queries = np.asarray(inputs["queries"], dtype=np.float32)
    keys = np.asarray(inputs["keys"], dtype=np.float32)
    mask = np.asarray(inputs["mask"]).astype(np.uint8)
    W_concat = np.asarray(inputs["W_concat"], dtype=np.float32)
    b_concat = np.asarray(inputs["b_concat"], dtype=np.float32)
    W_logit = np.asarray(inputs["W_logit"], dtype=np.float32)

    nc = _build_program()

    halves = NCORES // B  # 2
    wqT_d = np.ascontiguousarray(W_concat[:, :D].T.astype(np.float16))
    wkT_d = np.ascontiguousarray(W_concat[:, D:].T.astype(np.float16))
    b_cat = np.ascontiguousarray(b_concat.reshape(D, 1))
    w_log = np.ascontiguousarray(W_logit.reshape(D, 1))
    in_maps = []
    for c in range(NCORES):
        b, h = divmod(c, halves)
        in_maps.append(
            {
                "qT_sh": np.ascontiguousarray(queries[b, h * QSH : (h + 1) * QSH].T.astype(np.float16)),
                "kT_full": np.ascontiguousarray(keys[b].T.astype(np.float16)),
                "mask_sh": np.ascontiguousarray(mask[b, h * QSH : (h + 1) * QSH]),
                "wqT_d": wqT_d,
                "wkT_d": wkT_d,
                "b_cat": b_cat,
                "w_log": w_log,
            }
        )

    res = run_bass_kernel_spmd(
        nc, in_maps, core_ids=list(range(NCORES)), trace=trace
    )
    outs = [res.results[c]["out_w"] for c in range(NCORES)]
    full = np.concatenate(outs, axis=0).reshape(B, LQ, LKV)
    return full, res


def kernel(**inputs) -> np.ndarray:
    out, _ = _run(inputs, trace=False)
    return out
